# revision 1
# baseline (speedup 1.0000x reference)
"""CrossMerge kernel for trn2.

Math (per batch element):
    means_i = mean over C of g_i              (4, H, W)
    logits  = w_proj @ means + b_proj         (4, H, W)
    w       = softmax(logits, axis=0)         (4, H, W)
    out     = sum_i g_i * w_i                 (C, H, W)

Sharding: data-parallel over batch B=8 across 8 cores; weights replicated;
no cross-device communication.

Per-core layout: the 4 grids are host-stacked into gall (4, 256, 9216);
C=256 is split into 2 partition-chunks of 128.  Spatial axis tiled into
9 DMA tiles of 1024 cols (one 4 MB load + one 1 MB store each), each
split into 2 compute slices of 512 cols (fp32 PSUM bank width).

Per 512-col slice j the work is:
  PE  : 8 mm  logits L(4,512)  += ws_i(128,4)^T g_ic(128,512)   [fp32r]
        1 mm  S1(1,512) = ones4^T E         (softmax denominator)
        1 mm  R4(4,512) = broadcast R       (K=1)
        4 mm  Wb_i(128,512) = row-select broadcast of W4[i]
        8 mm  F_c(128,512) += I^T p_ic      (PSUM accumulation)
  ACT : E = exp(L + b)  [single table set];  3x copy Wb PSUM->SBUF
  DVE : R = reciprocal_approx_fast(S1);  W4 = E * R4;
        products for DVE grids; 2x F PSUM->out-tile copies
  POOL: products for remaining grids (SBUF operands only)

The d-loop is software-pipelined: products of iter d are issued in pass 1
of d, and the PE accumulation + output copy/store (pass 2) are emitted
before pass 1 of iter d+1, so the PE never sits idle waiting on the
product chain (keeps the HAM clock-gate at 8/8).

All narrow softmax tiles (L, S1, R4) share one PSUM bank at base
partitions 0/32/64 (the legal matmul output bases).

Codegen constraint honored throughout: TRN2 instructions support a single
sync wait; Bacc's generate_event_semaphores pass splits the rest.
"""

import os
import sys
from contextlib import ExitStack

import numpy as np

try:
    import concourse.bass as bass
except ImportError:  # fresh grading dir: concourse lives in the container repo
    sys.path.insert(0, "/opt/trn_rl_repo")
    import concourse.bass as bass

import concourse.tile as tile
from concourse import bacc, mybir
from concourse.bass_utils import run_bass_kernel_spmd

B, C, H, W = 8, 256, 96, 96
HW = H * W  # 9216
NCORES = 8
CPB = C // 128  # 2 partition chunks per core
DCOLS = 1024  # columns per DMA tile
JCOLS = 512  # columns per compute slice (= fp32 PSUM bank)
NDMA = HW // DCOLS  # 9
NJ = DCOLS // JCOLS  # 2

F32 = mybir.dt.float32
F32R = mybir.dt.float32r
AF = mybir.ActivationFunctionType


def dve_takes(c, i):
    """Product (chunk c, grid i) on DVE? Rest go to gpsimd."""
    return i in (0, 1) or (i == 2 and c == 0)


GPSIMD_GRIDS = (2, 3)  # grids needing an SBUF-staged weight copy

_CACHE = {}


def build_program():
    nc = bacc.Bacc("TRN2", debug=False, num_devices=NCORES)

    gall_d = nc.dram_tensor("gall", [4, C, HW], F32R, kind="ExternalInput").ap()
    # one blob for all constants -> single DMA, single semaphore lane.
    # cols: 0-15 ws | 16 bvec | 17 ones4 | 18-529 selmat | 530-657 ident
    #       | 658-661 ones1x4
    cb_d = nc.dram_tensor("cblob", [128, 662], F32R, kind="ExternalInput").ap()
    out = nc.dram_tensor("out", [C, HW], F32, kind="ExternalOutput").ap()

    with tile.TileContext(nc) as tc, ExitStack() as ctx:
        const = ctx.enter_context(tc.tile_pool(name="const", bufs=1))
        gin = ctx.enter_context(tc.tile_pool(name="gin", bufs=3))
        outp = ctx.enter_context(tc.tile_pool(name="outp", bufs=2))
        narrow = ctx.enter_context(tc.tile_pool(name="narrow", bufs=3))
        wbsb = ctx.enter_context(tc.tile_pool(name="wbsb", bufs=2))
        prod = ctx.enter_context(tc.tile_pool(name="prod", bufs=28))
        ps_smx = ctx.enter_context(tc.tile_pool(name="psmx", bufs=1, space="PSUM"))
        ps_S4 = ctx.enter_context(tc.tile_pool(name="psS4", bufs=1, space="PSUM"))
        ps_Wb = ctx.enter_context(tc.tile_pool(name="psWb", bufs=1, space="PSUM"))
        ps_F = ctx.enter_context(tc.tile_pool(name="psF", bufs=2, space="PSUM"))

        # constants -> SBUF in one DMA
        cb = const.tile([128, 662], F32R)
        nc.sync.dma_start(out=cb[:], in_=cb_d)
        ws = cb[:, 0:16]
        bv = cb[0:4, 16:17].bitcast(F32)
        ones4 = cb[0:4, 17:18]
        selmat = cb[0:4, 18:530]
        ident = cb[:, 530:658]
        ones4x4 = cb[0:4, 658:662].bitcast(F32)

        # Warmup matmul: absorbs the const-blob DMA wait on the PE clock.
        warm = ps_F.tile([4, 16], F32, tag="F")
        nc.tensor.matmul(warm[:], lhsT=ws[:, 0:4], rhs=ws, start=True, stop=True)

        def logits_exp(gat, j):
            """logits matmuls + fused bias-exp for slice j; returns E."""
            x0 = j * JCOLS
            smx = ps_smx.tile([128, JCOLS], F32)
            L = smx[0:4, :]
            k = 0
            for i in range(4):
                for c in range(CPB):
                    nc.tensor.matmul(
                        L,
                        lhsT=ws[:, 4 * i : 4 * i + 4],
                        rhs=gat[:, i, c, x0 : x0 + JCOLS],
                        start=(k == 0),
                        stop=(k == 7),
                    )
                    k += 1
            E = narrow.tile([4, JCOLS], F32R, tag="E")
            nc.scalar.activation(E[:], L, AF.Exp, bias=bv, scale=1.0)
            return E

        def denom(E):
            """S4 = row-replicated softmax denominator; R4 = 1/S4 (DVE).
            S4 sits at partition base 0: the custom reciprocal DVE op
            malfunctions at a nonzero base partition (HW-verified)."""
            S4 = ps_S4.tile([4, JCOLS], F32, tag="S4")
            nc.tensor.matmul(
                S4[:], lhsT=ones4x4.bitcast(F32R), rhs=E[:], start=True, stop=True
            )
            return S4

        def weights(E, S4):
            R4 = narrow.tile([4, JCOLS], F32, tag="R4")
            nc.vector.reciprocal_approx_fast(R4[:], S4[:])
            W4 = narrow.tile([4, JCOLS], F32R, tag="W4")
            nc.vector.tensor_mul(W4[:], E[:].bitcast(F32), R4[:])
            return W4

        def broadcast(W4):
            wbp, wbs = [], {}
            for i in range(4):
                Wbp = ps_Wb.tile([128, JCOLS], F32, tag=f"wb{i}")
                nc.tensor.matmul(
                    Wbp[:],
                    lhsT=selmat[:, 128 * i : 128 * (i + 1)],
                    rhs=W4[:],
                    start=True,
                    stop=True,
                )
                wbp.append(Wbp)
                if i in GPSIMD_GRIDS:
                    Wb = wbsb.tile([128, JCOLS], F32, tag=f"wbs{i}")
                    nc.scalar.copy(Wb[:], Wbp[:])
                    wbs[i] = Wb
            return wbp, wbs

        def products(gat, j, wbp, wbs, state):
            x0 = j * JCOLS
            for c in range(CPB):
                for i in range(4):
                    p = prod.tile([128, JCOLS], F32R, tag="p")
                    gslice = gat[:, i, c, x0 : x0 + JCOLS].bitcast(F32)
                    if dve_takes(c, i):
                        nc.vector.tensor_mul(p[:], gslice, wbp[i][:])
                    else:
                        nc.gpsimd.tensor_mul(p[:], gslice, wbs[i][:])
                    state[(j, c, i)] = p

        def accum_one(prev, j, c):
            """One PSUM accumulation group of the previous iter + its copy.
            Emitted between narrow-chain matmuls as PE gap filler."""
            if prev is None:
                return
            _, ot, state = prev
            x0 = j * JCOLS
            F = ps_F.tile([128, JCOLS], F32, tag="F")
            for i in range(4):
                nc.tensor.matmul(
                    F[:],
                    lhsT=ident,
                    rhs=state[(j, c, i)][:],
                    start=(i == 0),
                    stop=(i == 3),
                )
            nc.scalar.copy(ot[:, c, x0 : x0 + JCOLS], F[:])

        def store(prev):
            if prev is None:
                return
            d, ot, _ = prev
            n0 = d * DCOLS
            nc.sync.dma_start(
                out=out[:, n0 : n0 + DCOLS].rearrange("(c p) n -> p c n", c=CPB),
                in_=ot[:],
            )

        pp = None  # iter d-2: its accum groups interleave as PE gap fillers
        prev = None  # iter d-1: products still in flight
        for d in range(NDMA):
            n0 = d * DCOLS
            gat = gin.tile([128, 4, CPB, DCOLS], F32R, tag="gall")
            nc.sync.dma_start(
                out=gat[:],
                in_=gall_d[:, :, n0 : n0 + DCOLS].rearrange(
                    "i (c p) n -> p i c n", c=CPB
                ),
            )
            ot = outp.tile([128, CPB, DCOLS], F32)
            state = {}
            # interleaved emission: accumulation groups of iter d-2 fill the
            # PE gaps left by the softmax chain's cross-engine round trips,
            # and are two iterations behind so their products are always done
            E0 = logits_exp(gat, 0)
            accum_one(pp, 0, 0)
            E1 = logits_exp(gat, 1)
            accum_one(pp, 0, 1)
            S40 = denom(E0)
            W40 = weights(E0, S40)
            accum_one(pp, 1, 0)
            wbp0, wbs0 = broadcast(W40)
            products(gat, 0, wbp0, wbs0, state)
            S41 = denom(E1)
            W41 = weights(E1, S41)
            accum_one(pp, 1, 1)
            wbp1, wbs1 = broadcast(W41)
            products(gat, 1, wbp1, wbs1, state)
            store(pp)
            pp, prev = prev, (d, ot, state)
        for tail in (pp, prev):
            for j in range(NJ):
                for c in range(CPB):
                    accum_one(tail, j, c)
            store(tail)

    nc.compile()
    return nc


def _get_program():
    if "nc" not in _CACHE:
        _CACHE["nc"] = build_program()
    return _CACHE["nc"]



def make_cblob(w_proj, b_proj):
    w = np.asarray(w_proj, dtype=np.float32)
    b = np.asarray(b_proj, dtype=np.float32)
    ws = np.empty((128, 16), dtype=np.float32)
    for i in range(4):
        for o in range(4):
            ws[:, 4 * i + o] = w[o, i] / C
    cblob = np.zeros((128, 662), dtype=np.float32)
    cblob[:, 0:16] = ws
    cblob[0:4, 16] = b
    cblob[0:4, 17] = 1.0
    cblob[0:4, 18:530] = np.repeat(np.eye(4, dtype=np.float32), 128, axis=1)
    cblob[:, 530:658] = np.eye(128, dtype=np.float32)
    cblob[0:4, 658:662] = 1.0
    return cblob


LAST_RESULT = None


def kernel(g0, g1, g2, g3, w_proj, b_proj):
    global LAST_RESULT
    nc = _get_program()

    cblob = make_cblob(w_proj, b_proj)

    gall = np.stack(
        [np.asarray(x, dtype=np.float32).reshape(B, C, HW) for x in (g0, g1, g2, g3)],
        axis=1,
    )  # (B, 4, C, HW)
    in_maps = []
    for bi in range(NCORES):
        m = {"gall": np.ascontiguousarray(gall[bi]), "cblob": cblob}
        in_maps.append(m)

    res = run_bass_kernel_spmd(
        nc,
        in_maps,
        list(range(NCORES)),
        trace=bool(int(os.environ.get("CM_TRACE", "0"))),
        tmpdir=os.environ.get("CM_TRACE_DIR") or None,
    )
    LAST_RESULT = res
    out_full = np.stack(
        [res.results[bi]["out"].reshape(C, H, W) for bi in range(NCORES)], axis=0
    )
    return out_full



# revision 2
# speedup vs baseline: 1.3785x; 1.3785x over previous
"""CrossMerge kernel for trn2 — v2 (DMA-roofline oriented).

Math (per batch element):
    means_i = mean over C of g_i              (4, H, W)
    logits  = w_proj @ means + b_proj         (4, H, W)
    w       = softmax(logits, axis=0)         (4, H, W)
    out     = sum_i g_i * w_i                 (C, H, W)

Sharding: data-parallel over batch B=8 across 8 cores; weights replicated;
no cross-device communication.

v1 was Tensor-engine bound (42 matmuls per 1024-col tile, HAM oscillating
at K=4/8 for most of each iteration -> 226us vs the 132us HBM floor).
v2 moves the product-accumulate off the PE and the bulk dtype to bf16:

  - loads: SWDGE (gpsimd) DMAs cast fp32->bf16 in flight; SBUF tiles halve,
    HBM-side traffic unchanged (the binding resource: 47.2MB @ ~358GB/s).
  - PE per 512-col slice: 8 logits MMs (bf16, fp32 PSUM), 1 denominator,
    4 softmax-weight broadcasts = 13 MMs (~2.8us warm) -- far under the
    5.9us slice DMA time, so PE clock state no longer matters.
  - ACT: exp(L)+bias, and the 4 PSUM->SBUF bf16 copies of the broadcast
    weights.
  - DVE: reciprocal, W=E*R, then 8 bf16 products (2x mode) + add tree;
    the final adds write fp32 directly into the store tile.
  - stores: plain fp32 on the sync/HWDGE queue so they never head-block
    the load queue.

Tolerance is rel_err < 2e-2; bf16 internals land ~1e-3.
"""

import os
import sys
from contextlib import ExitStack

import numpy as np

try:
    import concourse.bass as bass
except ImportError:  # fresh grading dir: concourse lives in the container repo
    sys.path.insert(0, "/opt/trn_rl_repo")
    import concourse.bass as bass

import concourse.tile as tile
from concourse import bacc, mybir
from concourse.bass_utils import run_bass_kernel_spmd

B, C, H, W = 8, 256, 96, 96
HW = H * W  # 9216
NCORES = 8
CPB = C // 128  # 2 partition chunks per core
DCOLS = 512  # columns per DMA tile == per compute slice (fp32 PSUM bank)
NDMA = HW // DCOLS  # 18
OCOLS = 2 * DCOLS  # output store granularity (1MB fp32)

F32 = mybir.dt.float32
BF16 = mybir.dt.bfloat16
U16 = mybir.dt.uint16
AF = mybir.ActivationFunctionType

_CACHE = {}


def build_program():
    nc = bacc.Bacc("TRN2", debug=False, num_devices=NCORES)

    gall_d = nc.dram_tensor("gall", [4, C, HW], F32, kind="ExternalInput").ap()
    # bf16 constants, one blob: 0-15 ws | 16-19 ones4x4 | 20-531 selmat
    cbu_d = nc.dram_tensor("cbu", [128, 532], U16, kind="ExternalInput").ap()
    # fp32 constants: col 0 = exp bias (rows 0-3)
    cf_d = nc.dram_tensor("cf", [128, 1], F32, kind="ExternalInput").ap()
    out = nc.dram_tensor("out", [C, HW], F32, kind="ExternalOutput").ap()

    with tile.TileContext(nc) as tc, ExitStack() as ctx:
        const = ctx.enter_context(tc.tile_pool(name="const", bufs=1))
        gin = ctx.enter_context(tc.tile_pool(name="gin", bufs=8))
        outp = ctx.enter_context(tc.tile_pool(name="outp", bufs=2))
        narrow = ctx.enter_context(tc.tile_pool(name="narrow", bufs=3))
        wbsb = ctx.enter_context(tc.tile_pool(name="wbsb", bufs=2))
        prod = ctx.enter_context(tc.tile_pool(name="prod", bufs=2))
        ps_L = ctx.enter_context(tc.tile_pool(name="psL", bufs=2, space="PSUM"))
        ps_S4 = ctx.enter_context(tc.tile_pool(name="psS4", bufs=2, space="PSUM"))
        ps_Wb = ctx.enter_context(tc.tile_pool(name="psWb", bufs=1, space="PSUM"))

        cbu = const.tile([128, 532], U16)
        nc.sync.dma_start(out=cbu[:], in_=cbu_d)
        cb = cbu.bitcast(BF16)
        ws = cb[:, 0:16]
        ones4x4 = cb[0:4, 16:20]
        selmat = cb[0:4, 20:532]
        cf = const.tile([128, 1], F32)
        nc.sync.dma_start(out=cf[:], in_=cf_d)
        bv = cf[0:4, 0:1]

        def slice_compute(d, gat, ot, oj):
            # --- softmax chain for this 512-col slice ---
            L = ps_L.tile([4, DCOLS], F32, tag="L")
            k = 0
            for i in range(4):
                for c in range(CPB):
                    nc.tensor.matmul(
                        L,
                        lhsT=ws[:, 4 * i : 4 * i + 4],
                        rhs=gat[:, i, c, :],
                        start=(k == 0),
                        stop=(k == 7),
                    )
                    k += 1
            E = narrow.tile([4, DCOLS], BF16, tag="E")
            nc.scalar.activation(E[:], L, AF.Exp, bias=bv, scale=1.0)
            S4 = ps_S4.tile([4, DCOLS], F32, tag="S4")
            nc.tensor.matmul(S4[:], lhsT=ones4x4, rhs=E[:], start=True, stop=True)
            # reciprocal DVE op requires base partition 0 (HW-verified in v1)
            R4 = narrow.tile([4, DCOLS], F32, tag="R4")
            nc.vector.reciprocal_approx_fast(R4[:], S4[:])
            W4 = narrow.tile([4, DCOLS], BF16, tag="W4")
            nc.vector.tensor_mul(W4[:], E[:], R4[:])
            # --- broadcast weights to 128 partitions (PE) + bf16 copies ---
            wbs = []
            for i in range(4):
                Wbp = ps_Wb.tile([128, DCOLS], F32, tag=f"wb{i}")
                nc.tensor.matmul(
                    Wbp[:],
                    lhsT=selmat[:, 128 * i : 128 * (i + 1)],
                    rhs=W4[:],
                    start=True,
                    stop=True,
                )
                Wb = wbsb.tile([128, DCOLS], BF16, tag=f"ws{i}")
                nc.scalar.copy(Wb[:], Wbp[:])
                wbs.append(Wb)
            # --- products + accumulation tree (DVE, bf16 2x) ---
            for c in range(CPB):
                q0 = prod.tile([128, DCOLS], BF16, tag="q0")
                nc.vector.tensor_mul(q0[:], gat[:, 0, c, :], wbs[0][:])
                q1 = prod.tile([128, DCOLS], BF16, tag="q1")
                nc.vector.tensor_mul(q1[:], gat[:, 1, c, :], wbs[1][:])
                s01 = prod.tile([128, DCOLS], BF16, tag="s01")
                nc.vector.tensor_add(s01[:], q0[:], q1[:])
                q2 = prod.tile([128, DCOLS], BF16, tag="q2")
                nc.vector.tensor_mul(q2[:], gat[:, 2, c, :], wbs[2][:])
                q3 = prod.tile([128, DCOLS], BF16, tag="q3")
                nc.vector.tensor_mul(q3[:], gat[:, 3, c, :], wbs[3][:])
                s23 = prod.tile([128, DCOLS], BF16, tag="s23")
                nc.vector.tensor_add(s23[:], q2[:], q3[:])
                nc.vector.tensor_add(
                    ot[:, c, oj * DCOLS : (oj + 1) * DCOLS], s01[:], s23[:]
                )

        ot = None
        for d in range(NDMA):
            n0 = d * DCOLS
            gat = gin.tile([128, 4, CPB, DCOLS], BF16, tag="gall")
            # SWDGE cast-load: HBM fp32 -> SBUF bf16
            nc.gpsimd.dma_start(
                out=gat[:],
                in_=gall_d[:, :, n0 : n0 + DCOLS].rearrange(
                    "i (c p) n -> p i c n", c=CPB
                ),
            )
            if d % 2 == 0:
                ot = outp.tile([128, CPB, OCOLS], F32, tag="ot")
            slice_compute(d, gat, ot, d % 2)
            if d % 2 == 1:
                N0 = (d - 1) * DCOLS
                nc.sync.dma_start(
                    out=out[:, N0 : N0 + OCOLS].rearrange(
                        "(c p) n -> p c n", c=CPB
                    ),
                    in_=ot[:],
                )

    nc.compile()
    return nc


def _get_program():
    if "nc" not in _CACHE:
        _CACHE["nc"] = build_program()
    return _CACHE["nc"]


def _to_bf16_bits(x):
    """Round-to-nearest-even fp32 -> bf16 bit pattern (uint16)."""
    u = np.asarray(x, dtype=np.float32).view(np.uint32)
    rounded = u + 0x7FFF + ((u >> 16) & 1)
    return (rounded >> 16).astype(np.uint16)


def make_consts(w_proj, b_proj):
    w = np.asarray(w_proj, dtype=np.float32)
    b = np.asarray(b_proj, dtype=np.float32)
    ws = np.empty((128, 16), dtype=np.float32)
    for i in range(4):
        for o in range(4):
            ws[:, 4 * i + o] = w[o, i] / C
    cbu = np.zeros((128, 532), dtype=np.float32)
    cbu[:, 0:16] = ws
    cbu[0:4, 16:20] = 1.0
    cbu[0:4, 20:532] = np.repeat(np.eye(4, dtype=np.float32), 128, axis=1)
    cf = np.zeros((128, 1), dtype=np.float32)
    cf[0:4, 0] = b
    return _to_bf16_bits(cbu), cf


LAST_RESULT = None


def kernel(g0, g1, g2, g3, w_proj, b_proj):
    global LAST_RESULT
    nc = _get_program()

    cbu, cf = make_consts(w_proj, b_proj)

    gall = np.stack(
        [np.asarray(x, dtype=np.float32).reshape(B, C, HW) for x in (g0, g1, g2, g3)],
        axis=1,
    )  # (B, 4, C, HW)
    in_maps = []
    for bi in range(NCORES):
        m = {"gall": np.ascontiguousarray(gall[bi]), "cbu": cbu, "cf": cf}
        in_maps.append(m)

    res = run_bass_kernel_spmd(
        nc,
        in_maps,
        list(range(NCORES)),
        trace=bool(int(os.environ.get("CM_TRACE", "0"))),
        tmpdir=os.environ.get("CM_TRACE_DIR") or None,
    )
    LAST_RESULT = res
    out_full = np.stack(
        [res.results[bi]["out"].reshape(C, H, W) for bi in range(NCORES)], axis=0
    )
    return out_full
